# revision 1
# baseline (speedup 1.0000x reference)
"""GTLayer (graph transformer layer) distributed Bass kernel for 8 TRN2 cores.

Sharding: nodes (and their incoming edges) are partitioned across 8 cores by
node id (5000 dst nodes/core).  Host-side prep redistributes RAW input rows
per edge (the halo-exchange analog): for each core's dst-sorted, padded edge
list we build transposed per-edge arrays kT[e]=k[src_e], vT, qT(dst), efT.
The device does all model FLOPs: per-edge projections (Wk/Wv/Wq as stationary
matmul weights), edge-bias matmul, per-edge per-head dots (DVE), exp (ACT),
segment softmax-sum + weighted aggregation via one-hot matmuls into PSUM,
then Wo + residual + BN (global stats via AllReduce) + FFN + BN.
"""

import json
from contextlib import ExitStack
import numpy as np
import ml_dtypes

import concourse.bass as bass
import concourse.mybir as mybir
import concourse.tile as tile
from concourse.bass_utils import run_bass_kernel_spmd

bf16 = ml_dtypes.bfloat16

# problem constants (hardcoded per contract)
N, E, IN, H, D, ED = 40000, 640000, 128, 8, 16, 64
C = H * D            # 128
NCORE = 8
NSH = N // NCORE     # 5000 nodes per core
NG = 40              # node groups of <=128 per core (39*128+8)
SG_TILES = 20        # padded edge tiles per group (20*128 = 2560 slots)
SG = SG_TILES * 128
S = NG * SG          # slots per core
EPS = 1e-5

f32 = mybir.dt.float32
bft = mybir.dt.bfloat16


def _split_multiwaits_json(bir: bytes) -> bytes:
    """This walrus build allows only ONE sem wait per instruction; Tile emits
    multi-waits.  Split extras onto NoOps inserted before, same engine."""
    b = json.loads(bir)
    ctr = [0]
    changed = False
    for f in b.get("functions", []):
        for blk in f.get("blocks", []):
            insts = blk.get("instructions")
            if not insts:
                continue
            out = []
            for i in insts:
                si = i.get("sync_info")
                waits = (si or {}).get("on_wait") or []
                if len(waits) > 1:
                    changed = True
                    for w in waits[:-1]:
                        ctr[0] += 1
                        out.append({
                            "debug": i.get("debug", 0), "engine": i["engine"],
                            "ins": [], "name": f"I-wsplit-{ctr[0]}",
                            "opcode": "NoOp", "outs": [],
                            "text_hint": "wsplit",
                            "sync_info": {"on_update": [], "on_wait": [w]},
                        })
                    si["on_wait"] = [waits[-1]]
                out.append(i)
            blk["instructions"] = out
    return json.dumps(b).encode() if changed else bir


class _BassW(bass.Bass):
    def to_json_bytes(self) -> bytes:
        return _split_multiwaits_json(super().to_json_bytes())


def _build_program():
    nc = _BassW()
    dt_in = {
        "kT": (bft, [IN, S]), "vT": (bft, [IN, S]), "qeT": (bft, [IN, S]),
        "efT": (bft, [ED + 1, S]),
        "dstrel": (f32, [128, NG * SG_TILES]),
        "iota": (f32, [128, 128]),
        "qT": (f32, [IN, NSH]),
        "WkT": (bft, [IN, C]), "WvT": (bft, [IN, C]), "WqT": (bft, [IN, C]),
        "WeT": (bft, [ED + 1, H]),
        "WoT": (bft, [C, C]),
        "W1Ta": (bft, [C, C]), "W1Tb": (bft, [C, C]),
        "W2Ta": (bft, [C, C]), "W2Tb": (bft, [C, C]),
        "b1a": (f32, [128, 1]), "b1b": (f32, [128, 1]), "b2": (f32, [128, 1]),
        "g1": (f32, [128, 1]), "bt1": (f32, [128, 1]),
        "g2": (f32, [128, 1]), "bt2": (f32, [128, 1]),
    }
    dins = {k: nc.dram_tensor(k, sh, dt, kind="ExternalInput")
            for k, (dt, sh) in dt_in.items()}
    dout = nc.dram_tensor("out", [C, NSH], f32, kind="ExternalOutput")

    CH = 500  # phase-2 node chunk
    NCH = NSH // CH

    with tile.TileContext(nc) as tc:
        with (
            tc.tile_pool(name="const", bufs=1) as cpool,
            tc.tile_pool(name="wts", bufs=1) as wpool,
            tc.tile_pool(name="edge", bufs=2) as epool,
            tc.tile_pool(name="big", bufs=1) as bpool,
            tc.tile_pool(name="dram", bufs=1, space="DRAM") as dpool,
        ):
            # ---- constants / weights resident in SBUF ----
            iota_t = cpool.tile([128, 128], f32)
            nc.sync.dma_start(out=iota_t[:], in_=dins["iota"][:])
            w = {}
            for nm in ("WkT", "WvT", "WqT", "WoT", "W1Ta", "W1Tb", "W2Ta", "W2Tb"):
                w[nm] = wpool.tile([C, C], bft, name=nm, tag=nm)
                nc.sync.dma_start(out=w[nm][:], in_=dins[nm][:])
            we_t = wpool.tile([ED + 1, H], bft)
            nc.sync.dma_start(out=we_t[:], in_=dins["WeT"][:])
            vec = {}
            for nm in ("b1a", "b1b", "b2", "g1", "bt1", "g2", "bt2"):
                vec[nm] = wpool.tile([128, 1], f32, name=nm, tag=nm)
                nc.sync.dma_start(out=vec[nm][:], in_=dins[nm][:])
            qT_t = bpool.tile([IN, NSH], f32)
            nc.sync.dma_start(out=qT_t[:], in_=dins["qT"][:])

            # normalized aggregation output, channel-major, bf16
            aggT_sb = bpool.tile([C, NSH], bft)
            ident = cpool.tile([128, 128], bft)
            iota_col = cpool.tile([128, 1], mybir.dt.int32)
            nc.gpsimd.iota(iota_col[:], [[0, 1]], channel_multiplier=1)
            iota_col_f = cpool.tile([128, 1], f32)
            nc.vector.tensor_copy(iota_col_f[:], iota_col[:])
            nc.vector.tensor_tensor(
                out=ident[:], in0=iota_col_f[:].to_broadcast([128, 128]),
                in1=iota_t[:], op=mybir.AluOpType.is_equal)

            # ---- phase 1: per group ----
            ph1 = ExitStack()
            pspool = ph1.enter_context(tc.tile_pool(name="eps", bufs=1, space="PSUM"))
            aggpool = ph1.enter_context(tc.tile_pool(name="agg", bufs=1, space="PSUM"))
            for g in range(NG):
                n_lo = g * 128
                n_hi = min(NSH - n_lo, 128)
                agg_ps = aggpool.tile([128, C + H], f32)
                for t in range(SG_TILES):
                    e0 = g * SG + t * 128
                    kt = epool.tile([IN, 128], bft, tag="kt")
                    vt = epool.tile([IN, 128], bft, tag="vt")
                    qt = epool.tile([IN, 128], bft, tag="qt")
                    eft = epool.tile([ED + 1, 128], bft, tag="eft")
                    nc.sync.dma_start(out=kt[:], in_=dins["kT"][:, e0:e0 + 128])
                    nc.sync.dma_start(out=vt[:], in_=dins["vT"][:, e0:e0 + 128])
                    nc.sync.dma_start(out=qt[:], in_=dins["qeT"][:, e0:e0 + 128])
                    nc.sync.dma_start(out=eft[:], in_=dins["efT"][:, e0:e0 + 128])
                    dr = epool.tile([128, 1], f32, tag="dr")
                    nc.sync.dma_start(
                        out=dr[:], in_=dins["dstrel"][:, g * SG_TILES + t: g * SG_TILES + t + 1])

                    kp = pspool.tile([128, C], f32, tag="kp")
                    vp = pspool.tile([128, C], f32, tag="vp")
                    qp = pspool.tile([128, C], f32, tag="qp")
                    eb = pspool.tile([128, H], f32, tag="eb")
                    nc.tensor.matmul(kp[:], lhsT=kt[:], rhs=w["WkT"][:], start=True, stop=True)
                    nc.tensor.matmul(vp[:], lhsT=vt[:], rhs=w["WvT"][:], start=True, stop=True)
                    nc.tensor.matmul(qp[:], lhsT=qt[:], rhs=w["WqT"][:], start=True, stop=True)
                    nc.tensor.matmul(eb[:], lhsT=eft[:], rhs=we_t[:], start=True, stop=True)

                    # one-hot [e, n]
                    oh = epool.tile([128, 128], bft, tag="oh")
                    nc.vector.tensor_tensor(
                        out=oh[:], in0=dr[:].to_broadcast([128, 128]),
                        in1=iota_t[:], op=mybir.AluOpType.is_equal)

                    # scores
                    qps = epool.tile([128, C], f32, tag="qps")
                    nc.scalar.copy(qps[:], qp[:])
                    prod = epool.tile([128, C], f32, tag="prod")
                    nc.vector.tensor_tensor(out=prod[:], in0=kp[:], in1=qps[:],
                                            op=mybir.AluOpType.mult)
                    s0 = epool.tile([128, H], f32, tag="s0")
                    nc.vector.tensor_reduce(
                        out=s0[:], in_=prod[:].rearrange("p (h d) -> p h d", h=H),
                        axis=mybir.AxisListType.X, op=mybir.AluOpType.add)
                    sc = epool.tile([128, H], f32, tag="sc")
                    nc.vector.tensor_tensor(out=sc[:], in0=s0[:], in1=eb[:],
                                            op=mybir.AluOpType.add)
                    # rhs tile [Vw | ex]
                    rhs = epool.tile([128, C + H], bft, tag="rhs")
                    ex = rhs[:, C:C + H]
                    nc.scalar.activation(ex, sc[:], mybir.ActivationFunctionType.Exp)
                    nc.vector.tensor_tensor(
                        out=rhs[:, 0:C].rearrange("p (h d) -> p h d", h=H),
                        in0=vp[:].rearrange("p (h d) -> p h d", h=H),
                        in1=ex.to_broadcast([128, H, D]),
                        op=mybir.AluOpType.mult)
                    nc.tensor.matmul(agg_ps[:], lhsT=oh[:], rhs=rhs[:],
                                     start=(t == 0), stop=(t == SG_TILES - 1))

                # normalize by denominator and transpose to channel-major
                rec = epool.tile([128, H], f32, tag="rec")
                nc.vector.reciprocal(rec[:], agg_ps[:, C:C + H])
                aggn = epool.tile([128, C], bft, tag="aggn")
                nc.vector.tensor_tensor(
                    out=aggn[:].rearrange("p (h d) -> p h d", h=H),
                    in0=agg_ps[:, 0:C].rearrange("p (h d) -> p h d", h=H),
                    in1=rec[:].to_broadcast([128, H, D]),
                    op=mybir.AluOpType.mult)
                aggnT_ps = pspool.tile([128, 128], bft, tag="aggT")
                nc.tensor.transpose(aggnT_ps[:], aggn[:], ident[:])
                nc.vector.tensor_copy(aggT_sb[:, n_lo:n_lo + n_hi],
                                      aggnT_ps[:, 0:n_hi])

            ph1.close()
            # ---- phase 2: channel-major dense ----
            p2ctx = ExitStack()
            p2pool = p2ctx.enter_context(tc.tile_pool(name="ph2ps", bufs=1, space="PSUM"))
            rst = bpool.tile([C, NSH], f32)
            for ci in range(NCH):
                s0_ = ci * CH
                ps = p2pool.tile([128, CH], f32, tag="wo")
                nc.tensor.matmul(ps[:], lhsT=w["WoT"][:],
                                 rhs=aggT_sb[:, s0_:s0_ + CH], start=True, stop=True)
                nc.vector.tensor_tensor(out=rst[:, s0_:s0_ + CH], in0=ps[:],
                                        in1=qT_t[:, s0_:s0_ + CH],
                                        op=mybir.AluOpType.add)

            def bn_layer(x_sb, gv, btv, suffix):
                # global mean/var across all N nodes (AllReduce of sum/sumsq)
                st = bpool.tile([128, 2], f32, tag=f"st{suffix}")
                nc.vector.tensor_reduce(out=st[:, 0:1], in_=x_sb[:],
                                        axis=mybir.AxisListType.X,
                                        op=mybir.AluOpType.add)
                sq = bpool.tile([C, NSH], bft, tag="sqscratch")
                nc.scalar.activation(sq[:], x_sb[:],
                                     mybir.ActivationFunctionType.Square,
                                     accum_out=st[:, 1:2])
                bounce_in = dpool.tile([128, 2], f32, tag=f"bi{suffix}")
                bounce_out = dpool.tile([128, 2], f32, tag=f"bo{suffix}")
                nc.gpsimd.dma_start(out=bounce_in[:], in_=st[:])
                nc.gpsimd.collective_compute(
                    "AllReduce", mybir.AluOpType.add,
                    replica_groups=[list(range(NCORE))],
                    ins=[bounce_in.opt()], outs=[bounce_out.opt()])
                stg = bpool.tile([128, 2], f32, tag=f"stg{suffix}")
                nc.sync.dma_start(out=stg[:], in_=bounce_out[:])
                mean = bpool.tile([128, 1], f32, tag=f"mean{suffix}")
                nc.vector.tensor_scalar_mul(mean[:], stg[:, 0:1], 1.0 / N)
                msq = bpool.tile([128, 1], f32, tag=f"msq{suffix}")
                nc.scalar.activation(msq[:], mean[:],
                                     mybir.ActivationFunctionType.Square)
                var = bpool.tile([128, 1], f32, tag=f"var{suffix}")
                nc.vector.tensor_scalar_mul(var[:], stg[:, 1:2], 1.0 / N)
                nc.vector.tensor_tensor(out=var[:], in0=var[:], in1=msq[:],
                                        op=mybir.AluOpType.subtract)
                nc.vector.tensor_scalar_add(var[:], var[:], float(EPS))
                sd = bpool.tile([128, 1], f32, tag=f"sd{suffix}")
                nc.scalar.activation(sd[:], var[:],
                                     mybir.ActivationFunctionType.Sqrt)
                rsd = bpool.tile([128, 1], f32, tag=f"rsd{suffix}")
                nc.vector.reciprocal(rsd[:], sd[:])
                scale = bpool.tile([128, 1], f32, tag=f"scale{suffix}")
                nc.vector.tensor_tensor(out=scale[:], in0=rsd[:], in1=gv[:],
                                        op=mybir.AluOpType.mult)
                nmean = bpool.tile([128, 1], f32, tag=f"nm{suffix}")
                nc.vector.tensor_tensor(out=nmean[:], in0=mean[:], in1=scale[:],
                                        op=mybir.AluOpType.mult)
                shift = bpool.tile([128, 1], f32, tag=f"shift{suffix}")
                nc.vector.tensor_tensor(out=shift[:], in0=btv[:], in1=nmean[:],
                                        op=mybir.AluOpType.subtract)
                return scale, shift

            sc1, sh1 = bn_layer(rst, vec["g1"], vec["bt1"], "1")
            xbn = bpool.tile([C, NSH], f32)
            nc.scalar.activation(xbn[:], rst[:],
                                 mybir.ActivationFunctionType.Identity,
                                 bias=sh1[:], scale=sc1[:])
            xbn_bf = bpool.tile([C, NSH], bft)
            nc.vector.tensor_copy(xbn_bf[:], xbn[:])

            y = bpool.tile([C, NSH], f32)
            for ci in range(NCH):
                s0_ = ci * CH
                rhs2 = xbn_bf[:, s0_:s0_ + CH]
                h1a = p2pool.tile([128, CH], f32, tag="h1a")
                h1b = p2pool.tile([128, CH], f32, tag="h1b")
                nc.tensor.matmul(h1a[:], lhsT=w["W1Ta"][:], rhs=rhs2, start=True, stop=True)
                nc.tensor.matmul(h1b[:], lhsT=w["W1Tb"][:], rhs=rhs2, start=True, stop=True)
                r1a = epool.tile([128, CH], bft, tag="r1a")
                r1b = epool.tile([128, CH], bft, tag="r1b")
                nc.scalar.activation(r1a[:], h1a[:],
                                     mybir.ActivationFunctionType.Relu,
                                     bias=vec["b1a"][:])
                nc.scalar.activation(r1b[:], h1b[:],
                                     mybir.ActivationFunctionType.Relu,
                                     bias=vec["b1b"][:])
                h2 = p2pool.tile([128, CH], f32, tag="h2")
                nc.tensor.matmul(h2[:], lhsT=w["W2Ta"][:], rhs=r1a[:], start=True, stop=False)
                nc.tensor.matmul(h2[:], lhsT=w["W2Tb"][:], rhs=r1b[:], start=False, stop=True)
                # y = h2 + b2 + xbn
                yt = epool.tile([128, CH], f32, tag="yt")
                nc.scalar.activation(yt[:], h2[:],
                                     mybir.ActivationFunctionType.Identity,
                                     bias=vec["b2"][:])
                nc.vector.tensor_tensor(out=y[:, s0_:s0_ + CH], in0=yt[:],
                                        in1=xbn[:, s0_:s0_ + CH],
                                        op=mybir.AluOpType.add)

            sc2, sh2 = bn_layer(y, vec["g2"], vec["bt2"], "2")
            yout = bpool.tile([C, NSH], f32)
            nc.scalar.activation(yout[:], y[:],
                                 mybir.ActivationFunctionType.Identity,
                                 bias=sh2[:], scale=sc2[:])
            nc.sync.dma_start(out=dout[:], in_=yout[:])
            p2ctx.close()
    return nc


def _host_prep(q, k, v, edge_feat, src, dst, Wq, Wk, Wv, We, be, Wo,
               W1, b1, W2, b2, g1, bt1, g2, bt2):
    order = np.argsort(dst, kind="stable")
    src_s = src[order]
    dst_s = dst[order]
    ef_s = edge_feat[order]

    in_maps = []
    for m in range(NCORE):
        lo, hi = m * NSH, (m + 1) * NSH
        sel = (dst_s >= lo) & (dst_s < hi)
        srcm, dstm, efm = src_s[sel], dst_s[sel] - lo, ef_s[sel]
        # slot layout: per group g, SG slots
        kT = np.zeros((IN, S), dtype=bf16)
        vT = np.zeros((IN, S), dtype=bf16)
        qeT = np.zeros((IN, S), dtype=bf16)
        efT = np.zeros((ED + 1, S), dtype=bf16)
        dstrel = np.full((128, NG * SG_TILES), -1.0, dtype=np.float32)
        grp = dstm // 128
        for g in range(NG):
            gs = np.nonzero(grp == g)[0]
            ne = len(gs)
            assert ne <= SG, f"group {g} core {m} has {ne} edges > SG={SG}"
            base = g * SG
            kT[:, base:base + ne] = k[srcm[gs]].T
            vT[:, base:base + ne] = v[srcm[gs]].T
            qeT[:, base:base + ne] = q[dstm[gs] + lo].T
            efT[:ED, base:base + ne] = efm[gs].T
            efT[ED, base:base + ne] = 1.0
            rel = (dstm[gs] - g * 128).astype(np.float32)
            sl = np.arange(ne)
            dstrel[sl % 128, g * SG_TILES + sl // 128] = rel
        iota = np.broadcast_to(np.arange(128, dtype=np.float32), (128, 128)).copy()
        im = {
            "kT": kT, "vT": vT, "qeT": qeT, "efT": efT,
            "dstrel": dstrel, "iota": iota,
            "qT": q[lo:hi].T.astype(np.float32).copy(),
            "WkT": Wk.T.astype(bf16).copy(),
            "WvT": Wv.T.astype(bf16).copy(),
            "WqT": (Wq / np.sqrt(np.float32(D))).T.astype(bf16).copy(),
            "WeT": np.concatenate([We.T, be[None, :]], 0).astype(bf16).copy(),
            "WoT": Wo.T.astype(bf16).copy(),
            "W1Ta": W1[:C].T.astype(bf16).copy(),
            "W1Tb": W1[C:].T.astype(bf16).copy(),
            "W2Ta": W2.T[:C].astype(bf16).copy(),
            "W2Tb": W2.T[C:].astype(bf16).copy(),
            "b1a": b1[:C, None].astype(np.float32).copy(),
            "b1b": b1[C:, None].astype(np.float32).copy(),
            "b2": b2[:, None].astype(np.float32).copy(),
            "g1": g1[:, None].astype(np.float32).copy(),
            "bt1": bt1[:, None].astype(np.float32).copy(),
            "g2": g2[:, None].astype(np.float32).copy(),
            "bt2": bt2[:, None].astype(np.float32).copy(),
        }
        in_maps.append(im)
    return in_maps


RUN_KW = {}
LAST = {}


def kernel(**inputs):
    inputs = {k: np.asarray(v) for k, v in inputs.items()}
    in_maps = _host_prep(**inputs)
    nc = _build_program()
    res = run_bass_kernel_spmd(nc, in_maps, core_ids=list(range(NCORE)),
                               **RUN_KW)
    LAST["res"] = res
    out = np.concatenate([r["out"].T for r in res.results], axis=0)
    return out.astype(np.float32)



# revision 2
# speedup vs baseline: 1.1115x; 1.1115x over previous
"""GTLayer distributed Bass kernel v3 for 8 TRN2 cores.

Degree-aligned slot layout: per core, nodes sorted by in-degree, packed
into 40 groups of 128 (node rank = partition). Edge slot (g, t, p) = the
t-th in-edge of node ranked g*128+p. So:
  - qd for every tile of group g is just qd[group g] (partition-aligned);
  - segment aggregation = identity-stationary matmul accumulating tiles
    into PSUM (no one-hot build, no dst bookkeeping);
  - only 4.5% slot padding (vs 28% for fixed-size groups).
Host gathers RAW k/v rows per slot (fp8, channel-major); device projects
per-edge with Wk/Wv as moving operands, computes scores with aligned qd,
edge bias via 2-tile-stacked matmuls, softmax-aggregates, then
Wo+residual+BN(AllReduce)+FFN+BN as the baseline, on permuted node
order; the host inverts the permutation on output.
"""

import json
from contextlib import ExitStack
import numpy as np
import ml_dtypes

import concourse.bass as bass
import concourse.mybir as mybir
import concourse.tile as tile
from concourse.bass_utils import run_bass_kernel_spmd

bf16 = ml_dtypes.bfloat16
f8 = ml_dtypes.float8_e4m3

N, E, IN, H, D, ED = 40000, 640000, 128, 8, 16, 64
C = H * D
NCORE = 8
NSH = N // NCORE     # 5000
NG = 40
NPAD = NG * 128      # 5120
EPS = 1e-5

T_LIST = None  # set by _set_plan from the actual graph
U_LIST = None
cumT = cumU = None
TT = TU = S = None


def _set_plan(dst):
    """Derive per-group tile counts from the actual dst array."""
    global T_LIST, U_LIST, cumT, cumU, TT, TU, S
    tl = np.zeros((NCORE, NG), dtype=int)
    for m in range(NCORE):
        d = dst[(dst >= m * NSH) & (dst < (m + 1) * NSH)] - m * NSH
        deg = np.bincount(d, minlength=NSH)
        degs = np.sort(deg)[::-1]
        degs = np.concatenate([degs, np.zeros(NPAD - NSH, int)])
        for g in range(NG):
            tl[m, g] = degs[g * 128:(g + 1) * 128].max()
    T_LIST = [int(t) for t in np.maximum(tl.max(axis=0), 1)]
    U_LIST = [(t + 1) // 2 for t in T_LIST]
    cumT = np.concatenate([[0], np.cumsum(T_LIST)]).astype(int)
    cumU = np.concatenate([[0], np.cumsum(U_LIST)]).astype(int)
    TT = int(cumT[-1])
    TU = int(cumU[-1])
    S = TT * 128

f32 = mybir.dt.float32
bft = mybir.dt.bfloat16
fp8 = mybir.dt.float8e4


def _split_multiwaits_json(bir: bytes) -> bytes:
    b = json.loads(bir)
    ctr = [0]
    changed = False
    for f in b.get("functions", []):
        for blk in f.get("blocks", []):
            insts = blk.get("instructions")
            if not insts:
                continue
            out = []
            for i in insts:
                si = i.get("sync_info")
                waits = (si or {}).get("on_wait") or []
                if len(waits) > 1:
                    changed = True
                    for w in waits[:-1]:
                        ctr[0] += 1
                        out.append({
                            "debug": i.get("debug", 0), "engine": i["engine"],
                            "ins": [], "name": f"I-wsplit-{ctr[0]}",
                            "opcode": "NoOp", "outs": [],
                            "text_hint": "wsplit",
                            "sync_info": {"on_update": [], "on_wait": [w]},
                        })
                    si["on_wait"] = [waits[-1]]
                out.append(i)
            blk["instructions"] = out
    return json.dumps(b).encode() if changed else bir


class _BassW(bass.Bass):
    def to_json_bytes(self) -> bytes:
        return _split_multiwaits_json(super().to_json_bytes())


def _build_program():
    nc = _BassW()
    dt_in = {
        "kTe": (fp8, [IN, S]), "vTe": (bft, [IN, S]),
        "qshT": (bft, [IN, NPAD]),
        "qT": (f32, [IN, NSH]),
        "ef2": (bft, [128, TU * 128]),
        "maskT": (bft, [128, TT]),
        "beB": (f32, [128, 16]),
        "ident": (bft, [128, 128]),
        "WkT": (bft, [IN, C]), "WvT": (bft, [IN, C]), "WqT": (bft, [IN, C]),
        "We2": (bft, [128, 16]),
        "WoT": (bft, [C, C]),
        "W1Ta": (bft, [C, C]), "W1Tb": (bft, [C, C]),
        "W2Ta": (bft, [C, C]), "W2Tb": (bft, [C, C]),
        "b1a": (f32, [128, 1]), "b1b": (f32, [128, 1]), "b2": (f32, [128, 1]),
        "g1": (f32, [128, 1]), "bt1": (f32, [128, 1]),
        "g2": (f32, [128, 1]), "bt2": (f32, [128, 1]),
    }
    dins = {k: nc.dram_tensor(k, sh, dt, kind="ExternalInput")
            for k, (dt, sh) in dt_in.items()}
    dout = nc.dram_tensor("out", [C, NSH], f32, kind="ExternalOutput")

    TMAX = max(T_LIST)
    UMAX = max(U_LIST)
    CH = 500
    NCH = NSH // CH

    with tile.TileContext(nc) as tc:
        with (
            tc.tile_pool(name="wts", bufs=1) as wpool,
            tc.tile_pool(name="big", bufs=1) as bpool,
            tc.tile_pool(name="dram", bufs=1, space="DRAM") as dpool,
        ):
            w = {}
            for nm in ("WkT", "WvT", "WqT", "WoT", "W1Ta", "W1Tb",
                       "W2Ta", "W2Tb", "ident"):
                w[nm] = wpool.tile([128, 128], bft, name=nm, tag=nm)
                nc.sync.dma_start(out=w[nm][:], in_=dins[nm][:])
            we2 = wpool.tile([128, 16], bft)
            nc.sync.dma_start(out=we2[:], in_=dins["We2"][:])
            beB = wpool.tile([128, 16], f32)
            nc.sync.dma_start(out=beB[:], in_=dins["beB"][:])
            vec = {}
            for nm in ("b1a", "b1b", "b2", "g1", "bt1", "g2", "bt2"):
                vec[nm] = wpool.tile([128, 1], f32, name=nm, tag=nm)
                nc.sync.dma_start(out=vec[nm][:], in_=dins[nm][:])
            mk_sb = bpool.tile([128, TT], bft)
            nc.sync.dma_start(out=mk_sb[:], in_=dins["maskT"][:])

            # ---- phase 0: project qd (aligned with slot partitions) ----
            p0 = ExitStack()
            p0ps = p0.enter_context(tc.tile_pool(name="p0ps", bufs=2, space="PSUM"))
            p0sb = p0.enter_context(tc.tile_pool(name="p0sb", bufs=1))
            qsh = p0sb.tile([IN, NPAD], bft, tag="qsh")
            nc.sync.dma_start(out=qsh[:], in_=dins["qshT"][:])
            qd_sb = bpool.tile([128, NG * 128], bft, tag="qdsb")
            for c in range(NG):
                n0 = c * 128
                qps = p0ps.tile([128, 128], f32, tag="qp")
                nc.tensor.matmul(qps[:], lhsT=qsh[:, n0:n0 + 128],
                                 rhs=w["WqT"][:], start=True, stop=True)
                nc.vector.tensor_copy(qd_sb[:, n0:n0 + 128], qps[:])
            p0.close()

            aggT_sb = bpool.tile([C, NSH], bft, tag="aggT")

            # ---- phase 1 ----
            ph1 = ExitStack()
            epool = ph1.enter_context(tc.tile_pool(name="edge", bufs=2))
            kpool = ph1.enter_context(tc.tile_pool(name="kps", bufs=2, space="PSUM"))
            vpool = ph1.enter_context(tc.tile_pool(name="vps", bufs=2, space="PSUM"))
            ebpool = ph1.enter_context(tc.tile_pool(name="ebps", bufs=1, space="PSUM"))
            aggpool = ph1.enter_context(tc.tile_pool(name="aggps", bufs=2, space="PSUM"))
            tpool = ph1.enter_context(tc.tile_pool(name="tps", bufs=1, space="PSUM"))
            for g in range(NG):
                T = T_LIST[g]
                U = U_LIST[g]
                t0 = int(cumT[g])
                u0 = int(cumU[g])
                NCHK = (T + 3) // 4
                qd_g = qd_sb[:, g * 128:(g + 1) * 128]

                kte = epool.tile([IN, TMAX * 128], fp8, tag="kte")
                nc.sync.dma_start(out=kte[:, 0:T * 128],
                                  in_=dins["kTe"][:, t0 * 128:(t0 + T) * 128])
                ef_sb = epool.tile([128, UMAX * 128], bft, tag="ef")
                nc.sync.dma_start(out=ef_sb[:, 0:U * 128],
                                  in_=dins["ef2"][:, u0 * 128:(u0 + U) * 128])

                # edge bias (2 tiles per matmul) -> eb_sb [p, t*8+h]
                ebps = ebpool.tile([128, UMAX * 16], f32, tag="eb")
                for u in range(U):
                    nc.tensor.matmul(ebps[:, u * 16:(u + 1) * 16],
                                     lhsT=ef_sb[:, u * 128:(u + 1) * 128],
                                     rhs=we2[:], start=True, stop=True)
                eb_sb = epool.tile([128, UMAX * 16], f32, tag="ebs")
                nc.vector.tensor_tensor(
                    out=eb_sb[:, 0:U * 16].rearrange("p (u x) -> p u x", x=16),
                    in0=ebps[:, 0:U * 16].rearrange("p (u x) -> p u x", x=16),
                    in1=beB[:].rearrange("p (o x) -> p o x", o=1)
                        .to_broadcast([128, U, 16]),
                    op=mybir.AluOpType.add)

                # pass 1: project k per edge (4 tiles per PSUM bank), dot qd
                prod = epool.tile([128, TMAX * C], bft, tag="prod")
                for ch in range(NCHK):
                    tA = ch * 4
                    nt = min(4, T - tA)
                    kps = kpool.tile([128, 512], f32, tag="kp4")
                    for j in range(nt):
                        t = tA + j
                        nc.tensor.matmul(
                            kps[:, j * 128:(j + 1) * 128],
                            lhsT=kte[:, t * 128:(t + 1) * 128],
                            rhs=w["WkT"][:], start=True, stop=True)
                    nc.vector.tensor_tensor(
                        out=prod[:, tA * C:(tA + nt) * C]
                            .rearrange("p (t h d) -> p t h d", h=H, d=D),
                        in0=kps[:, 0:nt * 128]
                            .rearrange("p (t h d) -> p t h d", h=H, d=D),
                        in1=qd_g.rearrange("p (o h d) -> p o h d", o=1, d=D)
                            .to_broadcast([128, nt, H, D]),
                        op=mybir.AluOpType.mult)
                dots = epool.tile([128, TMAX * H], f32, tag="dots")
                nc.vector.tensor_reduce(
                    out=dots[:, 0:T * H].rearrange("p (t h) -> p t h", h=H),
                    in_=prod[:, 0:T * C].rearrange("p (t h d) -> p t h d",
                                                   h=H, d=D),
                    axis=mybir.AxisListType.X, op=mybir.AluOpType.add)
                nc.vector.tensor_tensor(
                    out=dots[:, 0:T * H], in0=dots[:, 0:T * H],
                    in1=eb_sb[:, 0:T * H], op=mybir.AluOpType.add)
                exb = epool.tile([128, TMAX * H], bft, tag="exb")
                nc.scalar.activation(exb[:, 0:T * H], dots[:, 0:T * H],
                                     mybir.ActivationFunctionType.Exp)
                vwx = epool.tile([128, TMAX * 136], bft, tag="vwx")
                vwx3 = vwx[:].rearrange("p (t q) -> p t q", q=136)
                nc.vector.tensor_tensor(
                    out=vwx3[:, 0:T, 128:136],
                    in0=exb[:, 0:T * H].rearrange("p (t h) -> p t h", h=H),
                    in1=mk_sb[:, t0:t0 + T]
                        .rearrange("p (t o) -> p t o", o=1)
                        .to_broadcast([128, T, H]),
                    op=mybir.AluOpType.mult)

                # pass 2: project v per edge, weight by ex
                vte = epool.tile([IN, TMAX * 128], bft, tag="vte")
                nc.sync.dma_start(out=vte[:, 0:T * 128],
                                  in_=dins["vTe"][:, t0 * 128:(t0 + T) * 128])
                for ch in range(NCHK):
                    tA = ch * 4
                    nt = min(4, T - tA)
                    vps = vpool.tile([128, 512], f32, tag="vp4")
                    for j in range(nt):
                        t = tA + j
                        nc.tensor.matmul(
                            vps[:, j * 128:(j + 1) * 128],
                            lhsT=vte[:, t * 128:(t + 1) * 128],
                            rhs=w["WvT"][:], start=True, stop=True)
                    nc.vector.tensor_tensor(
                        out=vwx3[:, tA:tA + nt, 0:128]
                            .rearrange("p t (h d) -> p t h d", d=D),
                        in0=vps[:, 0:nt * 128]
                            .rearrange("p (t h d) -> p t h d", h=H, d=D),
                        in1=vwx3[:, tA:tA + nt, 128:136]
                            .rearrange("p t (h o) -> p t h o", o=1)
                            .to_broadcast([128, nt, H, D]),
                        op=mybir.AluOpType.mult)

                # aggregate tiles (identity stationary, PSUM accumulate)
                aggps = aggpool.tile([128, 136], f32, tag="agg")
                for t in range(T):
                    nc.tensor.matmul(aggps[:], lhsT=w["ident"][:],
                                     rhs=vwx[:, t * 136:(t + 1) * 136],
                                     start=(t == 0), stop=(t == T - 1))

                dn = epool.tile([128, H], f32, tag="dn")
                nc.vector.tensor_scalar_add(dn[:], aggps[:, 128:136], 1e-30)
                rec = epool.tile([128, H], f32, tag="rec")
                nc.vector.reciprocal(rec[:], dn[:])
                aggn = epool.tile([128, 128], bft, tag="aggn")
                nc.vector.tensor_tensor(
                    out=aggn[:].rearrange("p (h d) -> p h d", d=D),
                    in0=aggps[:, 0:128].rearrange("p (h d) -> p h d", d=D),
                    in1=rec[:].rearrange("p (h o) -> p h o", o=1)
                        .to_broadcast([128, H, D]),
                    op=mybir.AluOpType.mult)
                aggT_ps = tpool.tile([128, 128], bft, tag="aggTp")
                nc.tensor.transpose(aggT_ps[:], aggn[:], w["ident"][:])
                n_hi = min(NSH - g * 128, 128)
                nc.vector.tensor_copy(aggT_sb[:, g * 128:g * 128 + n_hi],
                                      aggT_ps[:, 0:n_hi])
            ph1.close()

            # ---- phase 2 ----
            p2ctx = ExitStack()
            p2pool = p2ctx.enter_context(tc.tile_pool(name="ph2ps", bufs=2, space="PSUM"))
            p2sb = p2ctx.enter_context(tc.tile_pool(name="p2sb", bufs=1))
            qT_t = p2sb.tile([IN, NSH], f32, tag="qTt")
            nc.sync.dma_start(out=qT_t[:], in_=dins["qT"][:])
            rst = p2sb.tile([C, NSH], f32)
            for ci in range(NCH):
                s0_ = ci * CH
                ps = p2pool.tile([128, CH], f32, tag="wo")
                nc.tensor.matmul(ps[:], lhsT=w["WoT"][:],
                                 rhs=aggT_sb[:, s0_:s0_ + CH], start=True, stop=True)
                nc.vector.tensor_tensor(out=rst[:, s0_:s0_ + CH], in0=ps[:],
                                        in1=qT_t[:, s0_:s0_ + CH],
                                        op=mybir.AluOpType.add)

            def bn_layer(x_sb, gv, btv, suffix):
                st = p2sb.tile([128, 2], f32, tag=f"st{suffix}")
                nc.vector.tensor_reduce(out=st[:, 0:1], in_=x_sb[:],
                                        axis=mybir.AxisListType.X,
                                        op=mybir.AluOpType.add)
                sq = p2sb.tile([C, NSH], bft, tag="sqscratch")
                nc.scalar.activation(sq[:], x_sb[:],
                                     mybir.ActivationFunctionType.Square,
                                     accum_out=st[:, 1:2])
                bounce_in = dpool.tile([128, 2], f32, tag=f"bi{suffix}")
                bounce_out = dpool.tile([128, 2], f32, tag=f"bo{suffix}")
                nc.gpsimd.dma_start(out=bounce_in[:], in_=st[:])
                nc.gpsimd.collective_compute(
                    "AllReduce", mybir.AluOpType.add,
                    replica_groups=[list(range(NCORE))],
                    ins=[bounce_in.opt()], outs=[bounce_out.opt()])
                stg = p2sb.tile([128, 2], f32, tag=f"stg{suffix}")
                nc.sync.dma_start(out=stg[:], in_=bounce_out[:])
                mean = p2sb.tile([128, 1], f32, tag=f"mean{suffix}")
                nc.vector.tensor_scalar_mul(mean[:], stg[:, 0:1], 1.0 / N)
                msq = p2sb.tile([128, 1], f32, tag=f"msq{suffix}")
                nc.scalar.activation(msq[:], mean[:],
                                     mybir.ActivationFunctionType.Square)
                var = p2sb.tile([128, 1], f32, tag=f"var{suffix}")
                nc.vector.tensor_scalar_mul(var[:], stg[:, 1:2], 1.0 / N)
                nc.vector.tensor_tensor(out=var[:], in0=var[:], in1=msq[:],
                                        op=mybir.AluOpType.subtract)
                nc.vector.tensor_scalar_add(var[:], var[:], float(EPS))
                sd = p2sb.tile([128, 1], f32, tag=f"sd{suffix}")
                nc.scalar.activation(sd[:], var[:],
                                     mybir.ActivationFunctionType.Sqrt)
                rsd = p2sb.tile([128, 1], f32, tag=f"rsd{suffix}")
                nc.vector.reciprocal(rsd[:], sd[:])
                scale = p2sb.tile([128, 1], f32, tag=f"scale{suffix}")
                nc.vector.tensor_tensor(out=scale[:], in0=rsd[:], in1=gv[:],
                                        op=mybir.AluOpType.mult)
                nmean = p2sb.tile([128, 1], f32, tag=f"nm{suffix}")
                nc.vector.tensor_tensor(out=nmean[:], in0=mean[:], in1=scale[:],
                                        op=mybir.AluOpType.mult)
                shift = p2sb.tile([128, 1], f32, tag=f"shift{suffix}")
                nc.vector.tensor_tensor(out=shift[:], in0=btv[:], in1=nmean[:],
                                        op=mybir.AluOpType.subtract)
                return scale, shift

            sc1, sh1 = bn_layer(rst, vec["g1"], vec["bt1"], "1")
            xbn = p2sb.tile([C, NSH], f32)
            nc.scalar.activation(xbn[:], rst[:],
                                 mybir.ActivationFunctionType.Identity,
                                 bias=sh1[:], scale=sc1[:])
            xbn_bf = p2sb.tile([C, NSH], bft)
            nc.vector.tensor_copy(xbn_bf[:], xbn[:])

            y = p2sb.tile([C, NSH], f32)
            for ci in range(NCH):
                s0_ = ci * CH
                rhs2 = xbn_bf[:, s0_:s0_ + CH]
                h1a = p2pool.tile([128, CH], f32, tag="h1a")
                h1b = p2pool.tile([128, CH], f32, tag="h1b")
                nc.tensor.matmul(h1a[:], lhsT=w["W1Ta"][:], rhs=rhs2, start=True, stop=True)
                nc.tensor.matmul(h1b[:], lhsT=w["W1Tb"][:], rhs=rhs2, start=True, stop=True)
                r1a = p2sb.tile([128, CH], bft, tag="r1a")
                r1b = p2sb.tile([128, CH], bft, tag="r1b")
                nc.scalar.activation(r1a[:], h1a[:],
                                     mybir.ActivationFunctionType.Relu,
                                     bias=vec["b1a"][:])
                nc.scalar.activation(r1b[:], h1b[:],
                                     mybir.ActivationFunctionType.Relu,
                                     bias=vec["b1b"][:])
                h2 = p2pool.tile([128, CH], f32, tag="h2")
                nc.tensor.matmul(h2[:], lhsT=w["W2Ta"][:], rhs=r1a[:], start=True, stop=False)
                nc.tensor.matmul(h2[:], lhsT=w["W2Tb"][:], rhs=r1b[:], start=False, stop=True)
                yt = p2sb.tile([128, CH], f32, tag="yt")
                nc.scalar.activation(yt[:], h2[:],
                                     mybir.ActivationFunctionType.Identity,
                                     bias=vec["b2"][:])
                nc.vector.tensor_tensor(out=y[:, s0_:s0_ + CH], in0=yt[:],
                                        in1=xbn[:, s0_:s0_ + CH],
                                        op=mybir.AluOpType.add)

            sc2, sh2 = bn_layer(y, vec["g2"], vec["bt2"], "2")
            yout = p2sb.tile([C, NSH], f32)
            nc.scalar.activation(yout[:], y[:],
                                 mybir.ActivationFunctionType.Identity,
                                 bias=sh2[:], scale=sc2[:])
            nc.sync.dma_start(out=dout[:], in_=yout[:])
            p2ctx.close()
    return nc


def _host_prep(q, k, v, edge_feat, src, dst, Wq, Wk, Wv, We, be, Wo,
               W1, b1, W2, b2, g1, bt1, g2, bt2):
    eorder = np.argsort(dst, kind="stable")
    src_s = src[eorder]
    dst_s = dst[eorder]

    k8 = k.astype(f8)
    v8 = v.astype(bf16)

    in_maps = []
    perms = []
    for m in range(NCORE):
        lo, hi = m * NSH, (m + 1) * NSH
        sel = (dst_s >= lo) & (dst_s < hi)
        srcm = src_s[sel]
        efm = edge_feat[eorder[sel]]
        dstm = dst_s[sel] - lo
        deg = np.bincount(dstm, minlength=NSH)
        estart = np.concatenate([[0], np.cumsum(deg)]).astype(np.int64)
        order = np.argsort(-deg, kind="stable")
        deg_s = deg[order]
        perms.append(order)

        kTe = np.zeros((IN, S), dtype=f8)
        vTe = np.zeros((IN, S), dtype=bf16)
        maskT = np.zeros((128, TT), dtype=bf16)
        ef2 = np.zeros((128, TU * 128), dtype=bf16)
        # vectorized slot assignment: edge e (dst-sorted) -> slot of
        # (rank r = rank_of[dstm[e]], t = position within its node's edges)
        ne = len(srcm)
        rank_of = np.empty(NSH, dtype=np.int64)
        rank_of[order] = np.arange(NSH)
        r = rank_of[dstm]
        t_e = np.arange(ne, dtype=np.int64) - estart[dstm]
        gg = r // 128
        pp = r % 128
        assert (t_e < np.asarray(T_LIST)[gg]).all(), f"core {m}: plan overflow"
        ct = cumT[gg] + t_e
        col = ct * 128 + pp
        kTe[:, col] = k8[srcm].T
        vTe[:, col] = v8[srcm].T
        maskT[pp, ct] = 1
        ecol = (cumU[gg] + t_e // 2) * 128 + pp
        efT_all = efm.T.astype(bf16)
        even = (t_e % 2) == 0
        ef2[0:64, ecol[even]] = efT_all[:, even]
        ef2[64:128, ecol[~even]] = efT_all[:, ~even]

        qperm = np.zeros((NPAD, IN), dtype=np.float32)
        qperm[:NSH] = q[lo:hi][order]
        We2 = np.zeros((128, 16), dtype=bf16)
        We2[0:64, 0:8] = We.T.astype(bf16)
        We2[64:128, 8:16] = We.T.astype(bf16)
        beB = np.tile(be.astype(np.float32)[None, :], (128, 2))

        im = {
            "kTe": kTe, "vTe": vTe,
            "qshT": qperm.T.astype(bf16).copy(),
            "qT": qperm[:NSH].T.astype(np.float32).copy(),
            "ef2": ef2, "maskT": maskT, "beB": beB,
            "ident": np.eye(128, dtype=bf16),
            "WkT": Wk.T.astype(bf16).copy(),
            "WvT": Wv.T.astype(bf16).copy(),
            "WqT": (Wq / np.sqrt(np.float32(D))).T.astype(bf16).copy(),
            "We2": We2,
            "WoT": Wo.T.astype(bf16).copy(),
            "W1Ta": W1[:C].T.astype(bf16).copy(),
            "W1Tb": W1[C:].T.astype(bf16).copy(),
            "W2Ta": W2.T[:C].astype(bf16).copy(),
            "W2Tb": W2.T[C:].astype(bf16).copy(),
            "b1a": b1[:C, None].astype(np.float32).copy(),
            "b1b": b1[C:, None].astype(np.float32).copy(),
            "b2": b2[:, None].astype(np.float32).copy(),
            "g1": g1[:, None].astype(np.float32).copy(),
            "bt1": bt1[:, None].astype(np.float32).copy(),
            "g2": g2[:, None].astype(np.float32).copy(),
            "bt2": bt2[:, None].astype(np.float32).copy(),
        }
        in_maps.append(im)
    return in_maps, perms


RUN_KW = {}
LAST = {}


def kernel(**inputs):
    inputs = {kk: np.asarray(vv) for kk, vv in inputs.items()}
    _set_plan(inputs["dst"])
    in_maps, perms = _host_prep(**inputs)
    nc = _build_program()
    res = run_bass_kernel_spmd(nc, in_maps, core_ids=list(range(NCORE)),
                               **RUN_KW)
    LAST["res"] = res
    out = np.empty((N, C), dtype=np.float32)
    for m in range(NCORE):
        block = res.results[m]["out"].T.astype(np.float32)
        out[m * NSH + perms[m]] = block
    return out


# revision 7
# speedup vs baseline: 1.1240x; 1.0113x over previous
"""GTLayer distributed Bass kernel v3 for 8 TRN2 cores.

Degree-aligned slot layout: per core, nodes sorted by in-degree, packed
into 40 groups of 128 (node rank = partition). Edge slot (g, t, p) = the
t-th in-edge of node ranked g*128+p. So:
  - qd for every tile of group g is just qd[group g] (partition-aligned);
  - segment aggregation = identity-stationary matmul accumulating tiles
    into PSUM (no one-hot build, no dst bookkeeping);
  - only 4.5% slot padding (vs 28% for fixed-size groups).
Host gathers RAW k/v rows per slot (fp8, channel-major); device projects
per-edge with Wk/Wv as moving operands, computes scores with aligned qd,
edge bias via 2-tile-stacked matmuls, softmax-aggregates, then
Wo+residual+BN(AllReduce)+FFN+BN as the baseline, on permuted node
order; the host inverts the permutation on output.
"""

import json
from contextlib import ExitStack
import numpy as np
import ml_dtypes

import concourse.bass as bass
import concourse.mybir as mybir
import concourse.tile as tile
from concourse.bass_utils import run_bass_kernel_spmd

bf16 = ml_dtypes.bfloat16
f8 = ml_dtypes.float8_e4m3

N, E, IN, H, D, ED = 40000, 640000, 128, 8, 16, 64
C = H * D
NCORE = 8
NSH = N // NCORE     # 5000
NG = 40
NPAD = NG * 128      # 5120
EPS = 1e-5

T_LIST = None  # set by _set_plan from the actual graph
U_LIST = None
cumT = cumU = None
TT = TU = S = None


def _set_plan(dst):
    """Derive per-group tile counts from the actual dst array."""
    global T_LIST, U_LIST, cumT, cumU, TT, TU, S
    tl = np.zeros((NCORE, NG), dtype=int)
    for m in range(NCORE):
        d = dst[(dst >= m * NSH) & (dst < (m + 1) * NSH)] - m * NSH
        deg = np.bincount(d, minlength=NSH)
        degs = np.sort(deg)[::-1]
        degs = np.concatenate([degs, np.zeros(NPAD - NSH, int)])
        for g in range(NG):
            tl[m, g] = degs[g * 128:(g + 1) * 128].max()
    T_LIST = [int(t) for t in np.maximum(tl.max(axis=0), 1)]
    U_LIST = [(t + 1) // 2 for t in T_LIST]
    cumT = np.concatenate([[0], np.cumsum(T_LIST)]).astype(int)
    cumU = np.concatenate([[0], np.cumsum(U_LIST)]).astype(int)
    TT = int(cumT[-1])
    TU = int(cumU[-1])
    S = TT * 128

f32 = mybir.dt.float32
bft = mybir.dt.bfloat16
fp8 = mybir.dt.float8e4


def _split_multiwaits_json(bir: bytes) -> bytes:
    b = json.loads(bir)
    ctr = [0]
    changed = False
    for f in b.get("functions", []):
        for blk in f.get("blocks", []):
            insts = blk.get("instructions")
            if not insts:
                continue
            out = []
            for i in insts:
                si = i.get("sync_info")
                waits = (si or {}).get("on_wait") or []
                if len(waits) > 1:
                    changed = True
                    for w in waits[:-1]:
                        ctr[0] += 1
                        out.append({
                            "debug": i.get("debug", 0), "engine": i["engine"],
                            "ins": [], "name": f"I-wsplit-{ctr[0]}",
                            "opcode": "NoOp", "outs": [],
                            "text_hint": "wsplit",
                            "sync_info": {"on_update": [], "on_wait": [w]},
                        })
                    si["on_wait"] = [waits[-1]]
                out.append(i)
            blk["instructions"] = out
    return json.dumps(b).encode() if changed else bir


class _BassW(bass.Bass):
    def to_json_bytes(self) -> bytes:
        return _split_multiwaits_json(super().to_json_bytes())


def _build_program(sim_single=False):
    nc = _BassW()
    dt_in = {
        "kTe": (fp8, [IN, S]), "vTe": (bft, [IN, S]),
        "qshT": (bft, [IN, NPAD]),
        "qT": (f32, [IN, NSH]),
        "ef2": (bft, [128, TU * 128]),
        "maskT": (bft, [128, TT]),
        "beB": (f32, [128, 16]),
        "ident": (bft, [128, 128]),
        "WkT": (bft, [IN, C]), "WvT": (bft, [IN, C]), "WqT": (bft, [IN, C]),
        "We2": (bft, [128, 16]),
        "WoT": (bft, [C, C]),
        "W1Ta": (bft, [C, C]), "W1Tb": (bft, [C, C]),
        "W2Ta": (bft, [C, C]), "W2Tb": (bft, [C, C]),
        "b1a": (f32, [128, 1]), "b1b": (f32, [128, 1]), "b2": (f32, [128, 1]),
        "g1": (f32, [128, 1]), "bt1": (f32, [128, 1]),
        "g2": (f32, [128, 1]), "bt2": (f32, [128, 1]),
    }
    dins = {k: nc.dram_tensor(k, sh, dt, kind="ExternalInput")
            for k, (dt, sh) in dt_in.items()}
    dout = nc.dram_tensor("out", [C, NSH], f32, kind="ExternalOutput")

    TMAX = max(T_LIST)
    UMAX = max(U_LIST)
    CH = 500
    NCH = NSH // CH

    with tile.TileContext(nc) as tc:
        with (
            tc.tile_pool(name="wts", bufs=1) as wpool,
            tc.tile_pool(name="big", bufs=1) as bpool,
            tc.tile_pool(name="dram", bufs=1, space="DRAM") as dpool,
        ):
            w = {}
            for nm in ("WkT", "WvT", "WqT", "WoT", "W1Ta", "W1Tb",
                       "W2Ta", "W2Tb", "ident"):
                w[nm] = wpool.tile([128, 128], bft, name=nm, tag=nm)
                nc.sync.dma_start(out=w[nm][:], in_=dins[nm][:])
            we2 = wpool.tile([128, 16], bft)
            nc.sync.dma_start(out=we2[:], in_=dins["We2"][:])
            beB = wpool.tile([128, 16], f32)
            nc.sync.dma_start(out=beB[:], in_=dins["beB"][:])
            epsRhs = wpool.tile([128, 136], bft)
            nc.vector.memset(epsRhs[:, 0:128], 0.0)
            nc.vector.memset(epsRhs[:, 128:136], 1e-30)
            vec = {}
            for nm in ("b1a", "b1b", "b2", "g1", "bt1", "g2", "bt2"):
                vec[nm] = wpool.tile([128, 1], f32, name=nm, tag=nm)
                nc.sync.dma_start(out=vec[nm][:], in_=dins[nm][:])
            mk_sb = bpool.tile([128, TT], bft)
            nc.sync.dma_start(out=mk_sb[:], in_=dins["maskT"][:])

            # ---- phase 0: project qd (aligned with slot partitions) ----
            p0 = ExitStack()
            p0ps = p0.enter_context(tc.tile_pool(name="p0ps", bufs=2, space="PSUM"))
            p0sb = p0.enter_context(tc.tile_pool(name="p0sb", bufs=1))
            qsh = p0sb.tile([IN, NPAD], bft, tag="qsh")
            nc.sync.dma_start(out=qsh[:], in_=dins["qshT"][:])
            qd_sb = bpool.tile([128, NG * 128], bft, tag="qdsb")
            for c in range(NG):
                n0 = c * 128
                qps = p0ps.tile([128, 128], f32, tag="qp")
                nc.tensor.matmul(qps[:], lhsT=qsh[:, n0:n0 + 128],
                                 rhs=w["WqT"][:], start=True, stop=True)
                nc.scalar.copy(qd_sb[:, n0:n0 + 128], qps[:])
            p0.close()

            aggT_sb = bpool.tile([C, NSH], bft, tag="aggT")

            # ---- phase 1 ----
            ph1 = ExitStack()
            epool = ph1.enter_context(tc.tile_pool(name="edge", bufs=2))
            kpool = ph1.enter_context(tc.tile_pool(name="kps", bufs=1, space="PSUM"))
            vpool = ph1.enter_context(tc.tile_pool(name="vps", bufs=1, space="PSUM"))
            ebpool = ph1.enter_context(tc.tile_pool(name="ebps", bufs=1, space="PSUM"))
            aggpool = ph1.enter_context(tc.tile_pool(name="aggps", bufs=2, space="PSUM"))
            tpool = ph1.enter_context(tc.tile_pool(name="tps", bufs=1, space="PSUM"))
            for g in range(NG):
                T = T_LIST[g]
                U = U_LIST[g]
                t0 = int(cumT[g])
                u0 = int(cumU[g])
                NCHK = (T + 7) // 8
                qd_g = qd_sb[:, g * 128:(g + 1) * 128]

                kte = epool.tile([IN, TMAX * 128], fp8, tag="kte")
                nc.sync.dma_start(out=kte[:, 0:T * 128],
                                  in_=dins["kTe"][:, t0 * 128:(t0 + T) * 128])
                ef_sb = epool.tile([128, UMAX * 128], bft, tag="ef")
                nc.sync.dma_start(out=ef_sb[:, 0:U * 128],
                                  in_=dins["ef2"][:, u0 * 128:(u0 + U) * 128])

                # edge bias (2 tiles per matmul) -> eb_sb [p, t*8+h]
                ebps = ebpool.tile([128, UMAX * 16], f32, tag="eb")
                for u in range(U):
                    nc.tensor.matmul(ebps[:, u * 16:(u + 1) * 16],
                                     lhsT=ef_sb[:, u * 128:(u + 1) * 128],
                                     rhs=we2[:], start=True, stop=True)
                eb_sb = epool.tile([128, UMAX * 16], f32, tag="ebs")
                nc.vector.tensor_tensor(
                    out=eb_sb[:, 0:U * 16].rearrange("p (u x) -> p u x", x=16),
                    in0=ebps[:, 0:U * 16].rearrange("p (u x) -> p u x", x=16),
                    in1=beB[:].rearrange("p (o x) -> p o x", o=1)
                        .to_broadcast([128, U, 16]),
                    op=mybir.AluOpType.add)

                # pass 1: project k per edge (4 tiles per PSUM bank), dot qd
                prod = epool.tile([128, TMAX * C], bft, tag="prod")
                for ch in range(NCHK):
                    tA = ch * 8
                    nt = min(8, T - tA)
                    kps = kpool.tile([128, 1024], f32, tag="kp8")
                    for j in range(nt):
                        t = tA + j
                        nc.tensor.matmul(
                            kps[:, j * 128:(j + 1) * 128],
                            lhsT=kte[:, t * 128:(t + 1) * 128],
                            rhs=w["WkT"][:], start=True, stop=True)
                    nc.vector.tensor_tensor(
                        out=prod[:, tA * C:(tA + nt) * C]
                            .rearrange("p (t h d) -> p t h d", h=H, d=D),
                        in0=kps[:, 0:nt * 128]
                            .rearrange("p (t h d) -> p t h d", h=H, d=D),
                        in1=qd_g.rearrange("p (o h d) -> p o h d", o=1, d=D)
                            .to_broadcast([128, nt, H, D]),
                        op=mybir.AluOpType.mult)
                dots = epool.tile([128, TMAX * H], f32, tag="dots")
                nc.vector.tensor_reduce(
                    out=dots[:, 0:T * H].rearrange("p (t h) -> p t h", h=H),
                    in_=prod[:, 0:T * C].rearrange("p (t h d) -> p t h d",
                                                   h=H, d=D),
                    axis=mybir.AxisListType.X, op=mybir.AluOpType.add)
                nc.vector.tensor_tensor(
                    out=dots[:, 0:T * H], in0=dots[:, 0:T * H],
                    in1=eb_sb[:, 0:T * H], op=mybir.AluOpType.add)
                exb = epool.tile([128, TMAX * H], bft, tag="exb")
                nc.scalar.activation(exb[:, 0:T * H], dots[:, 0:T * H],
                                     mybir.ActivationFunctionType.Exp)
                vwx = epool.tile([128, TMAX * 136], bft, tag="vwx")
                vwx3 = vwx[:].rearrange("p (t q) -> p t q", q=136)
                nc.vector.tensor_tensor(
                    out=vwx3[:, 0:T, 128:136],
                    in0=exb[:, 0:T * H].rearrange("p (t h) -> p t h", h=H),
                    in1=mk_sb[:, t0:t0 + T]
                        .rearrange("p (t o) -> p t o", o=1)
                        .to_broadcast([128, T, H]),
                    op=mybir.AluOpType.mult)

                # pass 2: project v per edge, weight by ex
                vte = epool.tile([IN, TMAX * 128], bft, tag="vte")
                nc.sync.dma_start(out=vte[:, 0:T * 128],
                                  in_=dins["vTe"][:, t0 * 128:(t0 + T) * 128])
                for ch in range(NCHK):
                    tA = ch * 8
                    nt = min(8, T - tA)
                    vps = vpool.tile([128, 1024], f32, tag="vp8")
                    for j in range(nt):
                        t = tA + j
                        nc.tensor.matmul(
                            vps[:, j * 128:(j + 1) * 128],
                            lhsT=vte[:, t * 128:(t + 1) * 128],
                            rhs=w["WvT"][:], start=True, stop=True)
                    nc.vector.tensor_tensor(
                        out=vwx3[:, tA:tA + nt, 0:128]
                            .rearrange("p t (h d) -> p t h d", d=D),
                        in0=vps[:, 0:nt * 128]
                            .rearrange("p (t h d) -> p t h d", h=H, d=D),
                        in1=vwx3[:, tA:tA + nt, 128:136]
                            .rearrange("p t (h o) -> p t h o", o=1)
                            .to_broadcast([128, nt, H, D]),
                        op=mybir.AluOpType.mult)

                # aggregate tiles (identity stationary, PSUM accumulate)
                aggps = aggpool.tile([128, 136], f32, tag="agg")
                nc.tensor.matmul(aggps[:], lhsT=w["ident"][:],
                                 rhs=epsRhs[:], start=True, stop=False)
                for t in range(T):
                    nc.tensor.matmul(aggps[:], lhsT=w["ident"][:],
                                     rhs=vwx[:, t * 136:(t + 1) * 136],
                                     start=False, stop=(t == T - 1))

                rec = epool.tile([128, H], f32, tag="rec")
                nc.vector.reciprocal(rec[:], aggps[:, 128:136])
                aggn = epool.tile([128, 128], bft, tag="aggn")
                nc.vector.tensor_tensor(
                    out=aggn[:].rearrange("p (h d) -> p h d", d=D),
                    in0=aggps[:, 0:128].rearrange("p (h d) -> p h d", d=D),
                    in1=rec[:].rearrange("p (h o) -> p h o", o=1)
                        .to_broadcast([128, H, D]),
                    op=mybir.AluOpType.mult)
                aggT_ps = tpool.tile([128, 128], bft, tag="aggTp")
                nc.tensor.transpose(aggT_ps[:], aggn[:], w["ident"][:])
                n_hi = min(NSH - g * 128, 128)
                nc.scalar.copy(aggT_sb[:, g * 128:g * 128 + n_hi],
                               aggT_ps[:, 0:n_hi])
            ph1.close()

            # ---- phase 2 ----
            p2ctx = ExitStack()
            p2pool = p2ctx.enter_context(tc.tile_pool(name="ph2ps", bufs=2, space="PSUM"))
            p2sb = p2ctx.enter_context(tc.tile_pool(name="p2sb", bufs=1))
            qT_t = p2sb.tile([IN, NSH], f32, tag="qTt")
            nc.sync.dma_start(out=qT_t[:], in_=dins["qT"][:])
            rst = p2sb.tile([C, NSH], f32)
            for ci in range(NCH):
                s0_ = ci * CH
                ps = p2pool.tile([128, CH], f32, tag="wo")
                nc.tensor.matmul(ps[:], lhsT=w["WoT"][:],
                                 rhs=aggT_sb[:, s0_:s0_ + CH], start=True, stop=True)
                nc.vector.tensor_tensor(out=rst[:, s0_:s0_ + CH], in0=ps[:],
                                        in1=qT_t[:, s0_:s0_ + CH],
                                        op=mybir.AluOpType.add)

            def bn_layer(x_sb, gv, btv, suffix):
                st = p2sb.tile([128, 2], f32, tag=f"st{suffix}")
                nc.vector.tensor_reduce(out=st[:, 0:1], in_=x_sb[:],
                                        axis=mybir.AxisListType.X,
                                        op=mybir.AluOpType.add)
                sq = p2sb.tile([C, NSH], bft, tag="sqscratch")
                nc.scalar.activation(sq[:], x_sb[:],
                                     mybir.ActivationFunctionType.Square,
                                     accum_out=st[:, 1:2])
                stg = p2sb.tile([128, 2], f32, tag=f"stg{suffix}")
                if sim_single:
                    nc.vector.tensor_copy(stg[:], st[:])
                else:
                    bounce_in = dpool.tile([128, 2], f32, tag=f"bi{suffix}")
                    bounce_out = dpool.tile([128, 2], f32, tag=f"bo{suffix}")
                    nc.gpsimd.dma_start(out=bounce_in[:], in_=st[:])
                    nc.gpsimd.collective_compute(
                        "AllReduce", mybir.AluOpType.add,
                        replica_groups=[list(range(NCORE))],
                        ins=[bounce_in.opt()], outs=[bounce_out.opt()])
                    nc.sync.dma_start(out=stg[:], in_=bounce_out[:])
                mean = p2sb.tile([128, 1], f32, tag=f"mean{suffix}")
                nc.vector.tensor_scalar_mul(mean[:], stg[:, 0:1], 1.0 / N)
                msq = p2sb.tile([128, 1], f32, tag=f"msq{suffix}")
                nc.scalar.activation(msq[:], mean[:],
                                     mybir.ActivationFunctionType.Square)
                var = p2sb.tile([128, 1], f32, tag=f"var{suffix}")
                nc.vector.tensor_scalar_mul(var[:], stg[:, 1:2], 1.0 / N)
                nc.vector.tensor_tensor(out=var[:], in0=var[:], in1=msq[:],
                                        op=mybir.AluOpType.subtract)
                nc.vector.tensor_scalar_add(var[:], var[:], float(EPS))
                sd = p2sb.tile([128, 1], f32, tag=f"sd{suffix}")
                nc.scalar.activation(sd[:], var[:],
                                     mybir.ActivationFunctionType.Sqrt)
                rsd = p2sb.tile([128, 1], f32, tag=f"rsd{suffix}")
                nc.vector.reciprocal(rsd[:], sd[:])
                scale = p2sb.tile([128, 1], f32, tag=f"scale{suffix}")
                nc.vector.tensor_tensor(out=scale[:], in0=rsd[:], in1=gv[:],
                                        op=mybir.AluOpType.mult)
                nmean = p2sb.tile([128, 1], f32, tag=f"nm{suffix}")
                nc.vector.tensor_tensor(out=nmean[:], in0=mean[:], in1=scale[:],
                                        op=mybir.AluOpType.mult)
                shift = p2sb.tile([128, 1], f32, tag=f"shift{suffix}")
                nc.vector.tensor_tensor(out=shift[:], in0=btv[:], in1=nmean[:],
                                        op=mybir.AluOpType.subtract)
                return scale, shift

            sc1, sh1 = bn_layer(rst, vec["g1"], vec["bt1"], "1")
            xbn = p2sb.tile([C, NSH], f32)
            nc.scalar.activation(xbn[:], rst[:],
                                 mybir.ActivationFunctionType.Identity,
                                 bias=sh1[:], scale=sc1[:])
            xbn_bf = p2sb.tile([C, NSH], bft)
            nc.vector.tensor_copy(xbn_bf[:], xbn[:])

            y = p2sb.tile([C, NSH], f32)
            for ci in range(NCH):
                s0_ = ci * CH
                rhs2 = xbn_bf[:, s0_:s0_ + CH]
                h1a = p2pool.tile([128, CH], f32, tag="h1a")
                h1b = p2pool.tile([128, CH], f32, tag="h1b")
                nc.tensor.matmul(h1a[:], lhsT=w["W1Ta"][:], rhs=rhs2, start=True, stop=True)
                nc.tensor.matmul(h1b[:], lhsT=w["W1Tb"][:], rhs=rhs2, start=True, stop=True)
                r1a = p2sb.tile([128, CH], bft, tag="r1a")
                r1b = p2sb.tile([128, CH], bft, tag="r1b")
                nc.scalar.activation(r1a[:], h1a[:],
                                     mybir.ActivationFunctionType.Relu,
                                     bias=vec["b1a"][:])
                nc.scalar.activation(r1b[:], h1b[:],
                                     mybir.ActivationFunctionType.Relu,
                                     bias=vec["b1b"][:])
                h2 = p2pool.tile([128, CH], f32, tag="h2")
                nc.tensor.matmul(h2[:], lhsT=w["W2Ta"][:], rhs=r1a[:], start=True, stop=False)
                nc.tensor.matmul(h2[:], lhsT=w["W2Tb"][:], rhs=r1b[:], start=False, stop=True)
                yt = p2sb.tile([128, CH], f32, tag="yt")
                nc.scalar.activation(yt[:], h2[:],
                                     mybir.ActivationFunctionType.Identity,
                                     bias=vec["b2"][:])
                nc.vector.tensor_tensor(out=y[:, s0_:s0_ + CH], in0=yt[:],
                                        in1=xbn[:, s0_:s0_ + CH],
                                        op=mybir.AluOpType.add)

            sc2, sh2 = bn_layer(y, vec["g2"], vec["bt2"], "2")
            yout = p2sb.tile([C, NSH], f32)
            nc.scalar.activation(yout[:], y[:],
                                 mybir.ActivationFunctionType.Identity,
                                 bias=sh2[:], scale=sc2[:])
            nc.sync.dma_start(out=dout[:], in_=yout[:])
            p2ctx.close()
    return nc


def _host_prep(q, k, v, edge_feat, src, dst, Wq, Wk, Wv, We, be, Wo,
               W1, b1, W2, b2, g1, bt1, g2, bt2):
    eorder = np.argsort(dst, kind="stable")
    src_s = src[eorder]
    dst_s = dst[eorder]

    k8 = k.astype(f8)
    v8 = v.astype(bf16)

    in_maps = []
    perms = []
    for m in range(NCORE):
        lo, hi = m * NSH, (m + 1) * NSH
        sel = (dst_s >= lo) & (dst_s < hi)
        srcm = src_s[sel]
        efm = edge_feat[eorder[sel]]
        dstm = dst_s[sel] - lo
        deg = np.bincount(dstm, minlength=NSH)
        estart = np.concatenate([[0], np.cumsum(deg)]).astype(np.int64)
        order = np.argsort(-deg, kind="stable")
        deg_s = deg[order]
        perms.append(order)

        kTe = np.zeros((IN, S), dtype=f8)
        vTe = np.zeros((IN, S), dtype=bf16)
        maskT = np.zeros((128, TT), dtype=bf16)
        ef2 = np.zeros((128, TU * 128), dtype=bf16)
        # vectorized slot assignment: edge e (dst-sorted) -> slot of
        # (rank r = rank_of[dstm[e]], t = position within its node's edges)
        ne = len(srcm)
        rank_of = np.empty(NSH, dtype=np.int64)
        rank_of[order] = np.arange(NSH)
        r = rank_of[dstm]
        t_e = np.arange(ne, dtype=np.int64) - estart[dstm]
        gg = r // 128
        pp = r % 128
        assert (t_e < np.asarray(T_LIST)[gg]).all(), f"core {m}: plan overflow"
        ct = cumT[gg] + t_e
        col = ct * 128 + pp
        kTe[:, col] = k8[srcm].T
        vTe[:, col] = v8[srcm].T
        maskT[pp, ct] = 1
        ecol = (cumU[gg] + t_e // 2) * 128 + pp
        efT_all = efm.T.astype(bf16)
        even = (t_e % 2) == 0
        ef2[0:64, ecol[even]] = efT_all[:, even]
        ef2[64:128, ecol[~even]] = efT_all[:, ~even]

        qperm = np.zeros((NPAD, IN), dtype=np.float32)
        qperm[:NSH] = q[lo:hi][order]
        We2 = np.zeros((128, 16), dtype=bf16)
        We2[0:64, 0:8] = We.T.astype(bf16)
        We2[64:128, 8:16] = We.T.astype(bf16)
        beB = np.tile(be.astype(np.float32)[None, :], (128, 2))

        im = {
            "kTe": kTe, "vTe": vTe,
            "qshT": qperm.T.astype(bf16).copy(),
            "qT": qperm[:NSH].T.astype(np.float32).copy(),
            "ef2": ef2, "maskT": maskT, "beB": beB,
            "ident": np.eye(128, dtype=bf16),
            "WkT": Wk.T.astype(bf16).copy(),
            "WvT": Wv.T.astype(bf16).copy(),
            "WqT": (Wq / np.sqrt(np.float32(D))).T.astype(bf16).copy(),
            "We2": We2,
            "WoT": Wo.T.astype(bf16).copy(),
            "W1Ta": W1[:C].T.astype(bf16).copy(),
            "W1Tb": W1[C:].T.astype(bf16).copy(),
            "W2Ta": W2.T[:C].astype(bf16).copy(),
            "W2Tb": W2.T[C:].astype(bf16).copy(),
            "b1a": b1[:C, None].astype(np.float32).copy(),
            "b1b": b1[C:, None].astype(np.float32).copy(),
            "b2": b2[:, None].astype(np.float32).copy(),
            "g1": g1[:, None].astype(np.float32).copy(),
            "bt1": bt1[:, None].astype(np.float32).copy(),
            "g2": g2[:, None].astype(np.float32).copy(),
            "bt2": bt2[:, None].astype(np.float32).copy(),
        }
        in_maps.append(im)
    return in_maps, perms


RUN_KW = {}
LAST = {}


def kernel(**inputs):
    inputs = {kk: np.asarray(vv) for kk, vv in inputs.items()}
    _set_plan(inputs["dst"])
    in_maps, perms = _host_prep(**inputs)
    nc = _build_program()
    res = run_bass_kernel_spmd(nc, in_maps, core_ids=list(range(NCORE)),
                               **RUN_KW)
    LAST["res"] = res
    out = np.empty((N, C), dtype=np.float32)
    for m in range(NCORE):
        block = res.results[m]["out"].T.astype(np.float32)
        out[m * NSH + perms[m]] = block
    return out


# revision 9
# speedup vs baseline: 1.6421x; 1.4609x over previous
"""GTLayer distributed Bass kernel v3 for 8 TRN2 cores.

Degree-aligned slot layout: per core, nodes sorted by in-degree, packed
into 40 groups of 128 (node rank = partition). Edge slot (g, t, p) = the
t-th in-edge of node ranked g*128+p. So:
  - qd for every tile of group g is just qd[group g] (partition-aligned);
  - segment aggregation = identity-stationary matmul accumulating tiles
    into PSUM (no one-hot build, no dst bookkeeping);
  - only 4.5% slot padding (vs 28% for fixed-size groups).
Host gathers RAW k/v rows per slot (fp8, channel-major); device projects
per-edge with Wk/Wv as moving operands, computes scores with aligned qd,
edge bias via 2-tile-stacked matmuls, softmax-aggregates, then
Wo+residual+BN(AllReduce)+FFN+BN as the baseline, on permuted node
order; the host inverts the permutation on output.
"""

import json
from contextlib import ExitStack
import numpy as np
import ml_dtypes

import concourse.bass as bass
import concourse.mybir as mybir
import concourse.tile as tile
from concourse.bass_utils import run_bass_kernel_spmd

bf16 = ml_dtypes.bfloat16
f8 = ml_dtypes.float8_e4m3

N, E, IN, H, D, ED = 40000, 640000, 128, 8, 16, 64
C = H * D
NCORE = 8
NSH = N // NCORE     # 5000
NG = 40
NPAD = NG * 128      # 5120
EPS = 1e-5

T_LIST = None  # set by _set_plan from the actual graph
U_LIST = None
cumT = cumU = None
TT = TU = S = None


def _set_plan(dst):
    """Derive per-group tile counts from the actual dst array."""
    global T_LIST, U_LIST, cumT, cumU, TT, TU, S
    tl = np.zeros((NCORE, NG), dtype=int)
    for m in range(NCORE):
        d = dst[(dst >= m * NSH) & (dst < (m + 1) * NSH)] - m * NSH
        deg = np.bincount(d, minlength=NSH)
        degs = np.sort(deg)[::-1]
        degs = np.concatenate([degs, np.zeros(NPAD - NSH, int)])
        for g in range(NG):
            tl[m, g] = degs[g * 128:(g + 1) * 128].max()
    T_LIST = [int(t) for t in np.maximum(tl.max(axis=0), 1)]
    U_LIST = [(t + 1) // 2 for t in T_LIST]
    cumT = np.concatenate([[0], np.cumsum(T_LIST)]).astype(int)
    cumU = np.concatenate([[0], np.cumsum(U_LIST)]).astype(int)
    TT = int(cumT[-1])
    TU = int(cumU[-1])
    S = TT * 128

f32 = mybir.dt.float32
bft = mybir.dt.bfloat16
fp8 = mybir.dt.float8e4


def _split_multiwaits_json(bir: bytes) -> bytes:
    b = json.loads(bir)
    ctr = [0]
    changed = False
    for f in b.get("functions", []):
        for blk in f.get("blocks", []):
            insts = blk.get("instructions")
            if not insts:
                continue
            out = []
            for i in insts:
                si = i.get("sync_info")
                waits = (si or {}).get("on_wait") or []
                if len(waits) > 1:
                    changed = True
                    for w in waits[:-1]:
                        ctr[0] += 1
                        out.append({
                            "debug": i.get("debug", 0), "engine": i["engine"],
                            "ins": [], "name": f"I-wsplit-{ctr[0]}",
                            "opcode": "NoOp", "outs": [],
                            "text_hint": "wsplit",
                            "sync_info": {"on_update": [], "on_wait": [w]},
                        })
                    si["on_wait"] = [waits[-1]]
                out.append(i)
            blk["instructions"] = out
    return json.dumps(b).encode() if changed else bir


class _BassW(bass.Bass):
    def to_json_bytes(self) -> bytes:
        return _split_multiwaits_json(super().to_json_bytes())


def _build_program(sim_single=False):
    nc = _BassW()
    dt_in = {
        "kTe": (fp8, [IN, S]), "vTe": (bft, [IN, S]),
        "qshT": (bft, [IN, NPAD]),
        "qT": (f32, [IN, NSH]),
        "ef2": (bft, [128, TU * 128]),
        "maskT": (bft, [128, TT]),
        "beB": (f32, [128, 16]),
        "ident": (bft, [128, 128]),
        "WkT": (bft, [IN, C]), "WvT": (bft, [IN, C]), "WqT": (bft, [IN, C]),
        "We2": (bft, [128, 16]),
        "WoT": (bft, [C, C]),
        "W1Ta": (bft, [C, C]), "W1Tb": (bft, [C, C]),
        "W2Ta": (bft, [C, C]), "W2Tb": (bft, [C, C]),
        "b1a": (f32, [128, 1]), "b1b": (f32, [128, 1]), "b2": (f32, [128, 1]),
        "g1": (f32, [128, 1]), "bt1": (f32, [128, 1]),
        "g2": (f32, [128, 1]), "bt2": (f32, [128, 1]),
    }
    dins = {k: nc.dram_tensor(k, sh, dt, kind="ExternalInput")
            for k, (dt, sh) in dt_in.items()}
    dout = nc.dram_tensor("out", [C, NSH], f32, kind="ExternalOutput")

    TMAX = max(T_LIST)
    UMAX = max(U_LIST)
    CH = 500
    NCH = NSH // CH

    with tile.TileContext(nc) as tc:
        with (
            tc.tile_pool(name="wts", bufs=1) as wpool,
            tc.tile_pool(name="big", bufs=1) as bpool,
            tc.tile_pool(name="dram", bufs=1, space="DRAM") as dpool,
        ):
            w = {}
            for nm in ("WkT", "WvT", "WqT", "WoT", "W1Ta", "W1Tb",
                       "W2Ta", "W2Tb", "ident"):
                w[nm] = wpool.tile([128, 128], bft, name=nm, tag=nm)
                nc.sync.dma_start(out=w[nm][:], in_=dins[nm][:])
            we2 = wpool.tile([128, 16], bft)
            nc.sync.dma_start(out=we2[:], in_=dins["We2"][:])
            beB = wpool.tile([128, 16], f32)
            nc.sync.dma_start(out=beB[:], in_=dins["beB"][:])
            epsRhs = wpool.tile([128, 136], bft)
            nc.vector.memset(epsRhs[:, 0:128], 0.0)
            nc.vector.memset(epsRhs[:, 128:136], 1e-30)
            vec = {}
            for nm in ("b1a", "b1b", "b2", "g1", "bt1", "g2", "bt2"):
                vec[nm] = wpool.tile([128, 1], f32, name=nm, tag=nm)
                nc.sync.dma_start(out=vec[nm][:], in_=dins[nm][:])
            mk_sb = bpool.tile([128, TT], bft)
            nc.sync.dma_start(out=mk_sb[:], in_=dins["maskT"][:])

            # ---- phase 0: project qd (aligned with slot partitions) ----
            p0 = ExitStack()
            p0ps = p0.enter_context(tc.tile_pool(name="p0ps", bufs=2, space="PSUM"))
            p0sb = p0.enter_context(tc.tile_pool(name="p0sb", bufs=1))
            qsh = p0sb.tile([IN, NPAD], bft, tag="qsh")
            nc.sync.dma_start(out=qsh[:], in_=dins["qshT"][:])
            qd_sb = bpool.tile([128, NG * 128], bft, tag="qdsb")
            for c in range(NG):
                n0 = c * 128
                qps = p0ps.tile([128, 128], f32, tag="qp")
                nc.tensor.matmul(qps[:], lhsT=qsh[:, n0:n0 + 128],
                                 rhs=w["WqT"][:], start=True, stop=True)
                nc.scalar.copy(qd_sb[:, n0:n0 + 128], qps[:])
            p0.close()

            aggT_sb = bpool.tile([C, NSH], bft, tag="aggT")

            # ---- phase 1 ----
            ph1 = ExitStack()
            epool = ph1.enter_context(tc.tile_pool(name="edge", bufs=2))
            kpool = ph1.enter_context(tc.tile_pool(name="kps", bufs=1, space="PSUM"))
            vpool = ph1.enter_context(tc.tile_pool(name="vps", bufs=1, space="PSUM"))
            ebpool = ph1.enter_context(tc.tile_pool(name="ebps", bufs=1, space="PSUM"))
            aggpool = ph1.enter_context(tc.tile_pool(name="aggps", bufs=2, space="PSUM"))
            tpool = ph1.enter_context(tc.tile_pool(name="tps", bufs=1, space="PSUM"))
            for g in range(NG):
                T = T_LIST[g]
                U = U_LIST[g]
                t0 = int(cumT[g])
                u0 = int(cumU[g])
                NCHK = (T + 7) // 8
                qd_g = qd_sb[:, g * 128:(g + 1) * 128]

                kte = epool.tile([IN, TMAX * 128], fp8, tag="kte")
                nc.sync.dma_start(out=kte[:, 0:T * 128],
                                  in_=dins["kTe"][:, t0 * 128:(t0 + T) * 128])
                ef_sb = epool.tile([128, UMAX * 128], bft, tag="ef")
                nc.sync.dma_start(out=ef_sb[:, 0:U * 128],
                                  in_=dins["ef2"][:, u0 * 128:(u0 + U) * 128])

                # edge bias (2 tiles per matmul) -> eb_sb [p, t*8+h]
                ebps = ebpool.tile([128, UMAX * 16], f32, tag="eb")
                for u in range(U):
                    nc.tensor.matmul(ebps[:, u * 16:(u + 1) * 16],
                                     lhsT=ef_sb[:, u * 128:(u + 1) * 128],
                                     rhs=we2[:], start=True, stop=True)
                eb_sb = epool.tile([128, UMAX * 16], f32, tag="ebs")
                nc.vector.tensor_tensor(
                    out=eb_sb[:, 0:U * 16].rearrange("p (u x) -> p u x", x=16),
                    in0=ebps[:, 0:U * 16].rearrange("p (u x) -> p u x", x=16),
                    in1=beB[:].rearrange("p (o x) -> p o x", o=1)
                        .to_broadcast([128, U, 16]),
                    op=mybir.AluOpType.add)

                # pass 1: project k per edge (4 tiles per PSUM bank), dot qd
                prod = epool.tile([128, TMAX * C], bft, tag="prod")
                kpsb = epool.tile([128, TMAX * 128], bft, tag="kpsb")
                for ch in range(NCHK):
                    tA = ch * 8
                    nt = min(8, T - tA)
                    kps = kpool.tile([128, 1024], f32, tag="kp8")
                    for j in range(nt):
                        t = tA + j
                        nc.tensor.matmul(
                            kps[:, j * 128:(j + 1) * 128],
                            lhsT=kte[:, t * 128:(t + 1) * 128],
                            rhs=w["WkT"][:], start=True, stop=True)
                    nc.scalar.copy(kpsb[:, tA * 128:(tA + nt) * 128],
                                   kps[:, 0:nt * 128])
                    nc.vector.tensor_tensor(
                        out=prod[:, tA * C:(tA + nt) * C]
                            .rearrange("p (t h d) -> p t h d", h=H, d=D),
                        in0=kpsb[:, tA * 128:(tA + nt) * 128]
                            .rearrange("p (t h d) -> p t h d", h=H, d=D),
                        in1=qd_g.rearrange("p (o h d) -> p o h d", o=1, d=D)
                            .to_broadcast([128, nt, H, D]),
                        op=mybir.AluOpType.mult)
                dots = epool.tile([128, TMAX * H], f32, tag="dots")
                r1 = epool.tile([128, TMAX * 64], bft, tag="r1")
                r2 = epool.tile([128, TMAX * 32], bft, tag="r2")
                r3 = epool.tile([128, TMAX * 16], bft, tag="r3")
                pr4 = prod[:, 0:T * C].rearrange("p (th two d) -> p th two d",
                                                 two=2, d=8)
                nc.vector.tensor_tensor(
                    out=r1[:, 0:T * 64].rearrange("p (th d) -> p th d", d=8),
                    in0=pr4[:, :, 0, :], in1=pr4[:, :, 1, :],
                    op=mybir.AluOpType.add)
                r14 = r1[:, 0:T * 64].rearrange("p (th two d) -> p th two d",
                                                two=2, d=4)
                nc.vector.tensor_tensor(
                    out=r2[:, 0:T * 32].rearrange("p (th d) -> p th d", d=4),
                    in0=r14[:, :, 0, :], in1=r14[:, :, 1, :],
                    op=mybir.AluOpType.add)
                r24 = r2[:, 0:T * 32].rearrange("p (th two d) -> p th two d",
                                                two=2, d=2)
                nc.vector.tensor_tensor(
                    out=r3[:, 0:T * 16].rearrange("p (th d) -> p th d", d=2),
                    in0=r24[:, :, 0, :], in1=r24[:, :, 1, :],
                    op=mybir.AluOpType.add)
                r34 = r3[:, 0:T * 16].rearrange("p (th two) -> p th two", two=2)
                nc.vector.tensor_tensor(
                    out=dots[:, 0:T * H].rearrange("p (th o) -> p th o", o=1),
                    in0=r34[:, :, 0:1], in1=r34[:, :, 1:2],
                    op=mybir.AluOpType.add)
                nc.vector.tensor_tensor(
                    out=dots[:, 0:T * H], in0=dots[:, 0:T * H],
                    in1=eb_sb[:, 0:T * H], op=mybir.AluOpType.add)
                exb = epool.tile([128, TMAX * H], bft, tag="exb")
                nc.scalar.activation(exb[:, 0:T * H], dots[:, 0:T * H],
                                     mybir.ActivationFunctionType.Exp)
                vwx = epool.tile([128, TMAX * 136], bft, tag="vwx")
                vwx3 = vwx[:].rearrange("p (t q) -> p t q", q=136)
                nc.vector.tensor_tensor(
                    out=vwx3[:, 0:T, 128:136],
                    in0=exb[:, 0:T * H].rearrange("p (t h) -> p t h", h=H),
                    in1=mk_sb[:, t0:t0 + T]
                        .rearrange("p (t o) -> p t o", o=1)
                        .to_broadcast([128, T, H]),
                    op=mybir.AluOpType.mult)

                # pass 2: project v per edge, weight by ex
                vte = epool.tile([IN, TMAX * 128], bft, tag="vte")
                nc.sync.dma_start(out=vte[:, 0:T * 128],
                                  in_=dins["vTe"][:, t0 * 128:(t0 + T) * 128])
                vpsb = epool.tile([128, TMAX * 128], bft, tag="vpsb")
                for ch in range(NCHK):
                    tA = ch * 8
                    nt = min(8, T - tA)
                    vps = vpool.tile([128, 1024], f32, tag="vp8")
                    for j in range(nt):
                        t = tA + j
                        nc.tensor.matmul(
                            vps[:, j * 128:(j + 1) * 128],
                            lhsT=vte[:, t * 128:(t + 1) * 128],
                            rhs=w["WvT"][:], start=True, stop=True)
                    # evict to bf16 SBUF on the (idle) scalar engine so the
                    # vw multiply below runs at bf16 rate on DVE
                    nc.scalar.copy(vpsb[:, tA * 128:(tA + nt) * 128],
                                   vps[:, 0:nt * 128])
                    nc.vector.tensor_tensor(
                        out=vwx3[:, tA:tA + nt, 0:128]
                            .rearrange("p t (h d) -> p t h d", d=D),
                        in0=vpsb[:, tA * 128:(tA + nt) * 128]
                            .rearrange("p (t h d) -> p t h d", h=H, d=D),
                        in1=vwx3[:, tA:tA + nt, 128:136]
                            .rearrange("p t (h o) -> p t h o", o=1)
                            .to_broadcast([128, nt, H, D]),
                        op=mybir.AluOpType.mult)

                # aggregate tiles (identity stationary, PSUM accumulate)
                aggps = aggpool.tile([128, 136], f32, tag="agg")
                nc.tensor.matmul(aggps[:], lhsT=w["ident"][:],
                                 rhs=epsRhs[:], start=True, stop=False)
                for t in range(T):
                    nc.tensor.matmul(aggps[:], lhsT=w["ident"][:],
                                     rhs=vwx[:, t * 136:(t + 1) * 136],
                                     start=False, stop=(t == T - 1))

                rec = epool.tile([128, H], f32, tag="rec")
                nc.vector.reciprocal(rec[:], aggps[:, 128:136])
                aggn = epool.tile([128, 128], bft, tag="aggn")
                nc.vector.tensor_tensor(
                    out=aggn[:].rearrange("p (h d) -> p h d", d=D),
                    in0=aggps[:, 0:128].rearrange("p (h d) -> p h d", d=D),
                    in1=rec[:].rearrange("p (h o) -> p h o", o=1)
                        .to_broadcast([128, H, D]),
                    op=mybir.AluOpType.mult)
                aggT_ps = tpool.tile([128, 128], bft, tag="aggTp")
                nc.tensor.transpose(aggT_ps[:], aggn[:], w["ident"][:])
                n_hi = min(NSH - g * 128, 128)
                nc.scalar.copy(aggT_sb[:, g * 128:g * 128 + n_hi],
                               aggT_ps[:, 0:n_hi])
            ph1.close()

            # ---- phase 2 ----
            p2ctx = ExitStack()
            p2pool = p2ctx.enter_context(tc.tile_pool(name="ph2ps", bufs=2, space="PSUM"))
            p2sb = p2ctx.enter_context(tc.tile_pool(name="p2sb", bufs=1))
            qT_t = p2sb.tile([IN, NSH], f32, tag="qTt")
            nc.sync.dma_start(out=qT_t[:], in_=dins["qT"][:])
            rst = p2sb.tile([C, NSH], f32)
            for ci in range(NCH):
                s0_ = ci * CH
                ps = p2pool.tile([128, CH], f32, tag="wo")
                nc.tensor.matmul(ps[:], lhsT=w["WoT"][:],
                                 rhs=aggT_sb[:, s0_:s0_ + CH], start=True, stop=True)
                nc.vector.tensor_tensor(out=rst[:, s0_:s0_ + CH], in0=ps[:],
                                        in1=qT_t[:, s0_:s0_ + CH],
                                        op=mybir.AluOpType.add)

            def bn_layer(x_sb, gv, btv, suffix):
                st = p2sb.tile([128, 2], f32, tag=f"st{suffix}")
                nc.vector.tensor_reduce(out=st[:, 0:1], in_=x_sb[:],
                                        axis=mybir.AxisListType.X,
                                        op=mybir.AluOpType.add)
                sq = p2sb.tile([C, NSH], bft, tag="sqscratch")
                nc.scalar.activation(sq[:], x_sb[:],
                                     mybir.ActivationFunctionType.Square,
                                     accum_out=st[:, 1:2])
                stg = p2sb.tile([128, 2], f32, tag=f"stg{suffix}")
                if sim_single:
                    nc.vector.tensor_copy(stg[:], st[:])
                else:
                    bounce_in = dpool.tile([128, 2], f32, tag=f"bi{suffix}")
                    bounce_out = dpool.tile([128, 2], f32, tag=f"bo{suffix}")
                    nc.gpsimd.dma_start(out=bounce_in[:], in_=st[:])
                    nc.gpsimd.collective_compute(
                        "AllReduce", mybir.AluOpType.add,
                        replica_groups=[list(range(NCORE))],
                        ins=[bounce_in.opt()], outs=[bounce_out.opt()])
                    nc.sync.dma_start(out=stg[:], in_=bounce_out[:])
                mean = p2sb.tile([128, 1], f32, tag=f"mean{suffix}")
                nc.vector.tensor_scalar_mul(mean[:], stg[:, 0:1], 1.0 / N)
                msq = p2sb.tile([128, 1], f32, tag=f"msq{suffix}")
                nc.scalar.activation(msq[:], mean[:],
                                     mybir.ActivationFunctionType.Square)
                var = p2sb.tile([128, 1], f32, tag=f"var{suffix}")
                nc.vector.tensor_scalar_mul(var[:], stg[:, 1:2], 1.0 / N)
                nc.vector.tensor_tensor(out=var[:], in0=var[:], in1=msq[:],
                                        op=mybir.AluOpType.subtract)
                nc.vector.tensor_scalar_add(var[:], var[:], float(EPS))
                sd = p2sb.tile([128, 1], f32, tag=f"sd{suffix}")
                nc.scalar.activation(sd[:], var[:],
                                     mybir.ActivationFunctionType.Sqrt)
                rsd = p2sb.tile([128, 1], f32, tag=f"rsd{suffix}")
                nc.vector.reciprocal(rsd[:], sd[:])
                scale = p2sb.tile([128, 1], f32, tag=f"scale{suffix}")
                nc.vector.tensor_tensor(out=scale[:], in0=rsd[:], in1=gv[:],
                                        op=mybir.AluOpType.mult)
                nmean = p2sb.tile([128, 1], f32, tag=f"nm{suffix}")
                nc.vector.tensor_tensor(out=nmean[:], in0=mean[:], in1=scale[:],
                                        op=mybir.AluOpType.mult)
                shift = p2sb.tile([128, 1], f32, tag=f"shift{suffix}")
                nc.vector.tensor_tensor(out=shift[:], in0=btv[:], in1=nmean[:],
                                        op=mybir.AluOpType.subtract)
                return scale, shift

            sc1, sh1 = bn_layer(rst, vec["g1"], vec["bt1"], "1")
            xbn = p2sb.tile([C, NSH], f32)
            nc.scalar.activation(xbn[:], rst[:],
                                 mybir.ActivationFunctionType.Identity,
                                 bias=sh1[:], scale=sc1[:])
            xbn_bf = p2sb.tile([C, NSH], bft)
            nc.vector.tensor_copy(xbn_bf[:], xbn[:])

            y = p2sb.tile([C, NSH], f32)
            for ci in range(NCH):
                s0_ = ci * CH
                rhs2 = xbn_bf[:, s0_:s0_ + CH]
                h1a = p2pool.tile([128, CH], f32, tag="h1a")
                h1b = p2pool.tile([128, CH], f32, tag="h1b")
                nc.tensor.matmul(h1a[:], lhsT=w["W1Ta"][:], rhs=rhs2, start=True, stop=True)
                nc.tensor.matmul(h1b[:], lhsT=w["W1Tb"][:], rhs=rhs2, start=True, stop=True)
                r1a = p2sb.tile([128, CH], bft, tag="r1a")
                r1b = p2sb.tile([128, CH], bft, tag="r1b")
                nc.scalar.activation(r1a[:], h1a[:],
                                     mybir.ActivationFunctionType.Relu,
                                     bias=vec["b1a"][:])
                nc.scalar.activation(r1b[:], h1b[:],
                                     mybir.ActivationFunctionType.Relu,
                                     bias=vec["b1b"][:])
                h2 = p2pool.tile([128, CH], f32, tag="h2")
                nc.tensor.matmul(h2[:], lhsT=w["W2Ta"][:], rhs=r1a[:], start=True, stop=False)
                nc.tensor.matmul(h2[:], lhsT=w["W2Tb"][:], rhs=r1b[:], start=False, stop=True)
                yt = p2sb.tile([128, CH], f32, tag="yt")
                nc.scalar.activation(yt[:], h2[:],
                                     mybir.ActivationFunctionType.Identity,
                                     bias=vec["b2"][:])
                nc.vector.tensor_tensor(out=y[:, s0_:s0_ + CH], in0=yt[:],
                                        in1=xbn[:, s0_:s0_ + CH],
                                        op=mybir.AluOpType.add)

            sc2, sh2 = bn_layer(y, vec["g2"], vec["bt2"], "2")
            yout = p2sb.tile([C, NSH], f32)
            nc.scalar.activation(yout[:], y[:],
                                 mybir.ActivationFunctionType.Identity,
                                 bias=sh2[:], scale=sc2[:])
            nc.sync.dma_start(out=dout[:], in_=yout[:])
            p2ctx.close()
    return nc


def _host_prep(q, k, v, edge_feat, src, dst, Wq, Wk, Wv, We, be, Wo,
               W1, b1, W2, b2, g1, bt1, g2, bt2):
    eorder = np.argsort(dst, kind="stable")
    src_s = src[eorder]
    dst_s = dst[eorder]

    k8 = k.astype(f8)
    v8 = v.astype(bf16)

    in_maps = []
    perms = []
    for m in range(NCORE):
        lo, hi = m * NSH, (m + 1) * NSH
        sel = (dst_s >= lo) & (dst_s < hi)
        srcm = src_s[sel]
        efm = edge_feat[eorder[sel]]
        dstm = dst_s[sel] - lo
        deg = np.bincount(dstm, minlength=NSH)
        estart = np.concatenate([[0], np.cumsum(deg)]).astype(np.int64)
        order = np.argsort(-deg, kind="stable")
        deg_s = deg[order]
        perms.append(order)

        kTe = np.zeros((IN, S), dtype=f8)
        vTe = np.zeros((IN, S), dtype=bf16)
        maskT = np.zeros((128, TT), dtype=bf16)
        ef2 = np.zeros((128, TU * 128), dtype=bf16)
        # vectorized slot assignment: edge e (dst-sorted) -> slot of
        # (rank r = rank_of[dstm[e]], t = position within its node's edges)
        ne = len(srcm)
        rank_of = np.empty(NSH, dtype=np.int64)
        rank_of[order] = np.arange(NSH)
        r = rank_of[dstm]
        t_e = np.arange(ne, dtype=np.int64) - estart[dstm]
        gg = r // 128
        pp = r % 128
        assert (t_e < np.asarray(T_LIST)[gg]).all(), f"core {m}: plan overflow"
        ct = cumT[gg] + t_e
        col = ct * 128 + pp
        kTe[:, col] = k8[srcm].T
        vTe[:, col] = v8[srcm].T
        maskT[pp, ct] = 1
        ecol = (cumU[gg] + t_e // 2) * 128 + pp
        efT_all = efm.T.astype(bf16)
        even = (t_e % 2) == 0
        ef2[0:64, ecol[even]] = efT_all[:, even]
        ef2[64:128, ecol[~even]] = efT_all[:, ~even]

        qperm = np.zeros((NPAD, IN), dtype=np.float32)
        qperm[:NSH] = q[lo:hi][order]
        We2 = np.zeros((128, 16), dtype=bf16)
        We2[0:64, 0:8] = We.T.astype(bf16)
        We2[64:128, 8:16] = We.T.astype(bf16)
        beB = np.tile(be.astype(np.float32)[None, :], (128, 2))

        im = {
            "kTe": kTe, "vTe": vTe,
            "qshT": qperm.T.astype(bf16).copy(),
            "qT": qperm[:NSH].T.astype(np.float32).copy(),
            "ef2": ef2, "maskT": maskT, "beB": beB,
            "ident": np.eye(128, dtype=bf16),
            "WkT": Wk.T.astype(bf16).copy(),
            "WvT": Wv.T.astype(bf16).copy(),
            "WqT": (Wq / np.sqrt(np.float32(D))).T.astype(bf16).copy(),
            "We2": We2,
            "WoT": Wo.T.astype(bf16).copy(),
            "W1Ta": W1[:C].T.astype(bf16).copy(),
            "W1Tb": W1[C:].T.astype(bf16).copy(),
            "W2Ta": W2.T[:C].astype(bf16).copy(),
            "W2Tb": W2.T[C:].astype(bf16).copy(),
            "b1a": b1[:C, None].astype(np.float32).copy(),
            "b1b": b1[C:, None].astype(np.float32).copy(),
            "b2": b2[:, None].astype(np.float32).copy(),
            "g1": g1[:, None].astype(np.float32).copy(),
            "bt1": bt1[:, None].astype(np.float32).copy(),
            "g2": g2[:, None].astype(np.float32).copy(),
            "bt2": bt2[:, None].astype(np.float32).copy(),
        }
        in_maps.append(im)
    return in_maps, perms


RUN_KW = {}
LAST = {}


def kernel(**inputs):
    inputs = {kk: np.asarray(vv) for kk, vv in inputs.items()}
    _set_plan(inputs["dst"])
    in_maps, perms = _host_prep(**inputs)
    nc = _build_program()
    res = run_bass_kernel_spmd(nc, in_maps, core_ids=list(range(NCORE)),
                               **RUN_KW)
    LAST["res"] = res
    out = np.empty((N, C), dtype=np.float32)
    for m in range(NCORE):
        block = res.results[m]["out"].T.astype(np.float32)
        out[m * NSH + perms[m]] = block
    return out
